# revision 3
# baseline (speedup 1.0000x reference)
"""Erosion (5x5 sliding-window min, geodesic border pad 1e4) on TRN2.

Layout: partition p holds rows 8p..8p+7 of one image in segs 2..9 (one
contiguous 32KB/partition SWDGE cast-DMA f32->bf16 per image: 16 MiB
HBM reads/core, no halo re-read amplification). Halo rows (segs 0,1 =
prev partition's segs 8,9; segs 10,11 = next partition's segs 2,3) are
filled by two partition-shifted SBUF->SBUF HWDGE DMAs on the otherwise
idle sync engine, so they cost no HBM bandwidth and no gpsimd queue
time. Geodesic row pads (p0 segs 0,1 / p127 segs 10,11) are written
once per x buffer; the halo copies write partitions 1..127 / 0..126
only, so pads survive.

Compute is full-width bf16: vertical min (w2/w4/v cascade) entirely on
DVE; horizontal min (a/b cascade + o + edge columns) is split between
DVE (out segs 0..3) and GpSimd (out segs 4..7). DVE tensor_tensor runs
in single-read-port 2x mode and therefore never contends with GpSimd
for the shared SBUF port pair, so the two engines genuinely overlap.
Stores cast bf16->f32 in the DMA (SWDGE, 16 MiB writes/core).

Pipelining: the GpSimd engine queue is in-order (TTs and SWDGE
triggers share it), so per image the queue is ordered
[store g1 trigger, g2 horizontal TTs, store g2 trigger, load k+2
trigger]; 3 x-buffers / 2 out-buffers keep the SDMA queue fed. The
last image's g2 stores split in two for a shorter drain. bf16 keeps
rel err ~2e-3 (tolerance 2e-2).
"""

import numpy as np

import concourse.bacc as bacc
import concourse.mybir as mybir
import concourse.tile as tile
from concourse.bass import AP
from concourse.bass_utils import run_bass_kernel_spmd

B, H, W = 32, 1024, 1024
N_CORES = 8
PER_CORE = B // N_CORES     # 4 images per core
PX = 2
PAD_VAL = 1e4
F32 = mybir.dt.float32
BF16 = mybir.dt.bfloat16
MIN = mybir.AluOpType.min

KR = 8                      # output rows per partition (128*8 = 1024)
SEGS = KR + 2 * PX          # 12 segments per partition

_CACHE = {}


def build_nc(repeat: int = 1):
    nc = bacc.Bacc("TRN2", debug=False, num_devices=N_CORES)
    x = nc.dram_tensor("mask", [PER_CORE, H, W], F32, kind="ExternalInput").ap()
    y = nc.dram_tensor("out", [PER_CORE, H, W], F32, kind="ExternalOutput").ap()

    N = repeat * PER_CORE   # flat image stream

    with tile.TileContext(nc) as tc:
        with (
            tc.tile_pool(name="const", bufs=1) as cpool,
            tc.tile_pool(name="xp", bufs=1) as xpool,
            tc.tile_pool(name="wp", bufs=1) as wpool,
            tc.tile_pool(name="op", bufs=1) as opool,
        ):
            # 1e4 source for row-pad fills (memset can't start at
            # partition 127; DMA is exempt from start-partition rules)
            cpad = cpool.tile([128, PX * W], BF16)
            nc.vector.memset(cpad[:, :], PAD_VAL)

            # manual buffers; geodesic pad rows (p0 segs 0,1 / p127
            # segs 10,11) are written ONLY here, once per buffer
            xbufs, obufs = [], []
            for i in range(3):
                xb = xpool.tile([128, SEGS * W], BF16, tag=f"x{i}", name=f"xb{i}")
                nc.sync.dma_start(out=xb[0:1, 0 : PX * W], in_=cpad[0:1, :])
                nc.sync.dma_start(
                    out=xb[127:128, (SEGS - PX) * W : SEGS * W], in_=cpad[0:1, :]
                )
                xbufs.append(xb)
            for i in range(2):
                obufs.append(
                    opool.tile([128, KR * W], BF16, tag=f"o{i}", name=f"ob{i}")
                )
            w2 = wpool.tile([128, (SEGS - 2) * W], BF16, tag="w2")
            w2_3 = w2[:, :].rearrange("p (s c) -> p s c", s=SEGS - 2)
            v = wpool.tile([128, KR * W], BF16, tag="v")
            v3 = v[:, :].rearrange("p (s c) -> p s c", s=KR)
            aa = wpool.tile([128, KR * W], BF16, tag="a")
            a3 = aa[:, :].rearrange("p (s c) -> p s c", s=KR)
            bb = wpool.tile([128, KR * W], BF16, tag="b")
            b3 = bb[:, :].rearrange("p (s c) -> p s c", s=KR)

            def issue_load(k):
                """SWDGE cast load (f32->bf16): rows 8p..8p+7 -> segs
                2..9, one contiguous 32KB read per partition."""
                img = k % PER_CORE
                xb = xbufs[k % 3]
                nc.gpsimd.dma_start(
                    out=xb[0:128, PX * W : (PX + KR) * W],
                    in_=AP(
                        x.tensor, img * H * W, [[KR * W, 128], [1, KR * W]]
                    ),
                )

            def issue_halos(k):
                """Partition-shifted SBUF->SBUF halo copies (HWDGE on
                the sync engine; no HBM, no gpsimd queue time)."""
                xb = xbufs[k % 3]
                # segs 0,1 of p <- segs 8,9 of p-1 (p = 1..127)
                nc.sync.dma_start(
                    out=xb[1:128, 0 : PX * W],
                    in_=xb[0:127, KR * W : (KR + PX) * W],
                )
                # segs 10,11 of p <- segs 2,3 of p+1 (p = 0..126)
                nc.sync.dma_start(
                    out=xb[0:127, (PX + KR) * W : SEGS * W],
                    in_=xb[1:128, PX * W : 2 * PX * W],
                )

            def horizontal(eng, o3, sl):
                """a/b cascade + geodesic edge columns for out segs sl."""
                eng.tensor_tensor(
                    out=a3[:, sl, 0 : W - 1],
                    in0=v3[:, sl, 0 : W - 1],
                    in1=v3[:, sl, 1:W],
                    op=MIN,
                )
                eng.tensor_tensor(
                    out=b3[:, sl, 0 : W - 3],
                    in0=a3[:, sl, 0 : W - 3],
                    in1=a3[:, sl, 2 : W - 1],
                    op=MIN,
                )
                eng.tensor_tensor(
                    out=o3[:, sl, PX : W - PX],
                    in0=b3[:, sl, 0 : W - 2 * PX],
                    in1=v3[:, sl, 2 * PX : W],
                    op=MIN,
                )
                eng.tensor_tensor(
                    out=o3[:, sl, 0:1], in0=a3[:, sl, 0:1], in1=a3[:, sl, 1:2], op=MIN
                )
                eng.tensor_tensor(
                    out=o3[:, sl, 1:2], in0=b3[:, sl, 0:1], in1=v3[:, sl, 0:1], op=MIN
                )
                eng.tensor_tensor(
                    out=o3[:, sl, W - 2 : W - 1],
                    in0=b3[:, sl, W - 4 : W - 3],
                    in1=v3[:, sl, W - 2 : W - 1],
                    op=MIN,
                )
                eng.tensor_tensor(
                    out=o3[:, sl, W - 1 : W],
                    in0=a3[:, sl, W - 3 : W - 2],
                    in1=a3[:, sl, W - 2 : W - 1],
                    op=MIN,
                )

            def store(k, s0, nseg):
                """SWDGE cast store (bf16->f32): out segs s0..s0+nseg-1,
                partition p -> rows 8p+s0 .. (contiguous descriptors)."""
                img = k % PER_CORE
                ob = obufs[k % 2]
                nc.gpsimd.dma_start(
                    out=AP(
                        y.tensor,
                        img * H * W + s0 * W,
                        [[KR * W, 128], [1, nseg * W]],
                    ),
                    in_=ob[:, s0 * W : (s0 + nseg) * W],
                )

            # prologue: loads + halos for the first two stream positions
            issue_load(0)
            issue_halos(0)
            if N > 1:
                issue_load(1)
                issue_halos(1)

            for k in range(N):
                xb = xbufs[k % 3]
                ob = obufs[k % 2]
                x3 = xb[:, :].rearrange("p (s c) -> p s c", s=SEGS)
                o3 = ob[:, :].rearrange("p (s c) -> p s c", s=KR)

                # ---- group 1: out segs 0..3 (needs x segs 0..7) ----
                nc.vector.tensor_tensor(
                    out=w2_3[:, 0:7, :], in0=x3[:, 0:7, :], in1=x3[:, 1:8, :],
                    op=MIN,
                )
                # w4[0..4] in place (one extra for group 2's v[4])
                nc.vector.tensor_tensor(
                    out=w2_3[:, 0:5, :], in0=w2_3[:, 0:5, :], in1=w2_3[:, 2:7, :],
                    op=MIN,
                )
                nc.vector.tensor_tensor(
                    out=v3[:, 0:4, :], in0=w2_3[:, 0:4, :], in1=x3[:, 4:8, :],
                    op=MIN,
                )
                horizontal(nc.vector, o3, slice(0, 4))
                store(k, 0, 4)

                # ---- group 2: out segs 4..7 (adds x segs 8..11) ----
                # w2[7..9], w4[5..7], v[4..7] on DVE; horizontal on GP
                nc.vector.tensor_tensor(
                    out=w2_3[:, 7:10, :], in0=x3[:, 7:10, :], in1=x3[:, 8:11, :],
                    op=MIN,
                )
                nc.vector.tensor_tensor(
                    out=w2_3[:, 5:8, :], in0=w2_3[:, 5:8, :], in1=w2_3[:, 7:10, :],
                    op=MIN,
                )
                nc.vector.tensor_tensor(
                    out=v3[:, 4:8, :], in0=w2_3[:, 4:8, :], in1=x3[:, 8:12, :],
                    op=MIN,
                )
                if k == N - 1:
                    # shorter drain: two 2-seg pieces
                    horizontal(nc.vector, o3, slice(4, 6))
                    store(k, 4, 2)
                    horizontal(nc.vector, o3, slice(6, 8))
                    store(k, 6, 2)
                else:
                    horizontal(nc.vector, o3, slice(4, 8))
                    store(k, 4, 4)

                # loads + halos for stream position k+2 AFTER this
                # image's stores (keeps the in-order queues flowing)
                if k + 2 < N:
                    issue_load(k + 2)
                    issue_halos(k + 2)

    nc.compile()
    return nc


def run(mask: np.ndarray, trace: bool = False, tmpdir: str | None = None):
    assert mask.shape == (B, 1, H, W), mask.shape
    in_dtype = mask.dtype
    mask4 = np.ascontiguousarray(
        mask.reshape(B, H, W).astype(np.float32, copy=False)
    )
    if "nc" not in _CACHE:
        _CACHE["nc"] = build_nc(1)
    nc = _CACHE["nc"]
    in_maps = [
        {"mask": mask4[i * PER_CORE : (i + 1) * PER_CORE]} for i in range(N_CORES)
    ]
    res = run_bass_kernel_spmd(
        nc, in_maps, list(range(N_CORES)), trace=trace, tmpdir=tmpdir
    )
    out = np.concatenate([res.results[i]["out"] for i in range(N_CORES)], axis=0)
    return out.reshape(B, 1, H, W).astype(in_dtype, copy=False), res


def kernel(mask: np.ndarray) -> np.ndarray:
    return run(mask)[0]


# revision 4
# speedup vs baseline: 1.5212x; 1.5212x over previous
"""Erosion (5x5 sliding-window min, geodesic border pad 1e4) on TRN2.

Layout: partition p holds rows 8p..8p+7 of one image in segs 2..9 (one
contiguous 32KB/partition SWDGE cast-DMA f32->bf16 per image: 16 MiB
HBM reads/core, no halo re-read amplification). Halo rows (segs 0,1 =
prev partition's segs 8,9; segs 10,11 = next partition's segs 2,3) are
filled by two partition-shifted SBUF->SBUF SWDGE DMAs (HWDGE chokes
on 4KB-run descriptor generation), costing no HBM bandwidth. Geodesic row pads (p0 segs 0,1 / p127 segs 10,11) are written
once per x buffer; the halo copies write partitions 1..127 / 0..126
only, so pads survive.

Compute is full-width bf16: vertical min (w2/w4/v cascade) entirely on
DVE; horizontal min (a/b cascade + o + edge columns) is split between
DVE (out segs 0..3) and GpSimd (out segs 4..7). DVE tensor_tensor runs
in single-read-port 2x mode and therefore never contends with GpSimd
for the shared SBUF port pair, so the two engines genuinely overlap.
Stores cast bf16->f32 in the DMA (SWDGE, 16 MiB writes/core).

Pipelining: the GpSimd engine queue is in-order (TTs and SWDGE
triggers share it), so per image the queue is ordered
[store g1 trigger, g2 horizontal TTs, store g2 trigger, load k+2
trigger]; 3 x-buffers / 2 out-buffers keep the SDMA queue fed. The
last image's g2 stores split in two for a shorter drain. bf16 keeps
rel err ~2e-3 (tolerance 2e-2).
"""

import numpy as np

import concourse.bacc as bacc
import concourse.mybir as mybir
import concourse.tile as tile
from concourse.bass import AP
from concourse.bass_utils import run_bass_kernel_spmd

B, H, W = 32, 1024, 1024
N_CORES = 8
PER_CORE = B // N_CORES     # 4 images per core
PX = 2
PAD_VAL = 1e4
F32 = mybir.dt.float32
BF16 = mybir.dt.bfloat16
MIN = mybir.AluOpType.min

KR = 8                      # output rows per partition (128*8 = 1024)
SEGS = KR + 2 * PX          # 12 segments per partition

_CACHE = {}


def build_nc(repeat: int = 1):
    nc = bacc.Bacc("TRN2", debug=False, num_devices=N_CORES)
    x = nc.dram_tensor("mask", [PER_CORE, H, W], F32, kind="ExternalInput").ap()
    y = nc.dram_tensor("out", [PER_CORE, H, W], F32, kind="ExternalOutput").ap()

    N = repeat * PER_CORE   # flat image stream

    with tile.TileContext(nc) as tc:
        with (
            tc.tile_pool(name="const", bufs=1) as cpool,
            tc.tile_pool(name="xp", bufs=1) as xpool,
            tc.tile_pool(name="wp", bufs=1) as wpool,
            tc.tile_pool(name="op", bufs=1) as opool,
        ):
            # 1e4 source for row-pad fills (memset can't start at
            # partition 127; DMA is exempt from start-partition rules)
            cpad = cpool.tile([128, PX * W], BF16)
            nc.vector.memset(cpad[:, :], PAD_VAL)

            # manual buffers; geodesic pad rows (p0 segs 0,1 / p127
            # segs 10,11) are written ONLY here, once per buffer
            xbufs, obufs = [], []
            for i in range(3):
                xb = xpool.tile([128, SEGS * W], BF16, tag=f"x{i}", name=f"xb{i}")
                nc.sync.dma_start(out=xb[0:1, 0 : PX * W], in_=cpad[0:1, :])
                nc.sync.dma_start(
                    out=xb[127:128, (SEGS - PX) * W : SEGS * W], in_=cpad[0:1, :]
                )
                xbufs.append(xb)
            for i in range(2):
                obufs.append(
                    opool.tile([128, KR * W], BF16, tag=f"o{i}", name=f"ob{i}")
                )
            w2 = wpool.tile([128, (SEGS - 2) * W], BF16, tag="w2")
            w2_3 = w2[:, :].rearrange("p (s c) -> p s c", s=SEGS - 2)
            v = wpool.tile([128, KR * W], BF16, tag="v")
            v3 = v[:, :].rearrange("p (s c) -> p s c", s=KR)
            aa = wpool.tile([128, KR * W], BF16, tag="a")
            a3 = aa[:, :].rearrange("p (s c) -> p s c", s=KR)
            bb = wpool.tile([128, KR * W], BF16, tag="b")
            b3 = bb[:, :].rearrange("p (s c) -> p s c", s=KR)

            def issue_load(k):
                """SWDGE cast load (f32->bf16): rows 8p..8p+7 -> segs
                2..9, one contiguous 32KB read per partition."""
                img = k % PER_CORE
                xb = xbufs[k % 3]
                nc.gpsimd.dma_start(
                    out=xb[0:128, PX * W : (PX + KR) * W],
                    in_=AP(
                        x.tensor, img * H * W, [[KR * W, 128], [1, KR * W]]
                    ),
                )

            def issue_halos(k):
                """Partition-shifted SBUF->SBUF halo copies (HWDGE on
                the sync engine; no HBM, no gpsimd queue time)."""
                xb = xbufs[k % 3]
                # segs 0,1 of p <- segs 8,9 of p-1 (p = 1..127).
                # SWDGE, not HWDGE: hardware descriptor generation
                # stalls ~60ns+/descriptor on 4KB-run layouts, which
                # gated compute by ~25us/image when these were on sync.
                nc.gpsimd.dma_start(
                    out=xb[1:128, 0 : PX * W],
                    in_=xb[0:127, KR * W : (KR + PX) * W],
                )
                # segs 10,11 of p <- segs 2,3 of p+1 (p = 0..126)
                nc.gpsimd.dma_start(
                    out=xb[0:127, (PX + KR) * W : SEGS * W],
                    in_=xb[1:128, PX * W : 2 * PX * W],
                )

            def horizontal(eng, o3, sl):
                """a/b cascade + geodesic edge columns for out segs sl."""
                eng.tensor_tensor(
                    out=a3[:, sl, 0 : W - 1],
                    in0=v3[:, sl, 0 : W - 1],
                    in1=v3[:, sl, 1:W],
                    op=MIN,
                )
                eng.tensor_tensor(
                    out=b3[:, sl, 0 : W - 3],
                    in0=a3[:, sl, 0 : W - 3],
                    in1=a3[:, sl, 2 : W - 1],
                    op=MIN,
                )
                eng.tensor_tensor(
                    out=o3[:, sl, PX : W - PX],
                    in0=b3[:, sl, 0 : W - 2 * PX],
                    in1=v3[:, sl, 2 * PX : W],
                    op=MIN,
                )
                eng.tensor_tensor(
                    out=o3[:, sl, 0:1], in0=a3[:, sl, 0:1], in1=a3[:, sl, 1:2], op=MIN
                )
                eng.tensor_tensor(
                    out=o3[:, sl, 1:2], in0=b3[:, sl, 0:1], in1=v3[:, sl, 0:1], op=MIN
                )
                eng.tensor_tensor(
                    out=o3[:, sl, W - 2 : W - 1],
                    in0=b3[:, sl, W - 4 : W - 3],
                    in1=v3[:, sl, W - 2 : W - 1],
                    op=MIN,
                )
                eng.tensor_tensor(
                    out=o3[:, sl, W - 1 : W],
                    in0=a3[:, sl, W - 3 : W - 2],
                    in1=a3[:, sl, W - 2 : W - 1],
                    op=MIN,
                )

            def store(k, s0, nseg):
                """SWDGE cast store (bf16->f32): out segs s0..s0+nseg-1,
                partition p -> rows 8p+s0 .. (contiguous descriptors)."""
                img = k % PER_CORE
                ob = obufs[k % 2]
                nc.gpsimd.dma_start(
                    out=AP(
                        y.tensor,
                        img * H * W + s0 * W,
                        [[KR * W, 128], [1, nseg * W]],
                    ),
                    in_=ob[:, s0 * W : (s0 + nseg) * W],
                )

            # prologue: loads + halos for the first two stream positions
            issue_load(0)
            issue_halos(0)
            if N > 1:
                issue_load(1)
                issue_halos(1)

            for k in range(N):
                xb = xbufs[k % 3]
                ob = obufs[k % 2]
                x3 = xb[:, :].rearrange("p (s c) -> p s c", s=SEGS)
                o3 = ob[:, :].rearrange("p (s c) -> p s c", s=KR)

                # ---- group 1: out segs 0..3 (needs x segs 0..7) ----
                nc.vector.tensor_tensor(
                    out=w2_3[:, 0:7, :], in0=x3[:, 0:7, :], in1=x3[:, 1:8, :],
                    op=MIN,
                )
                # w4[0..4] in place (one extra for group 2's v[4])
                nc.vector.tensor_tensor(
                    out=w2_3[:, 0:5, :], in0=w2_3[:, 0:5, :], in1=w2_3[:, 2:7, :],
                    op=MIN,
                )
                nc.vector.tensor_tensor(
                    out=v3[:, 0:4, :], in0=w2_3[:, 0:4, :], in1=x3[:, 4:8, :],
                    op=MIN,
                )
                horizontal(nc.vector, o3, slice(0, 4))
                store(k, 0, 4)

                # ---- group 2: out segs 4..7 (adds x segs 8..11) ----
                # w2[7..9], w4[5..7], v[4..7] on DVE; horizontal on GP
                nc.vector.tensor_tensor(
                    out=w2_3[:, 7:10, :], in0=x3[:, 7:10, :], in1=x3[:, 8:11, :],
                    op=MIN,
                )
                nc.vector.tensor_tensor(
                    out=w2_3[:, 5:8, :], in0=w2_3[:, 5:8, :], in1=w2_3[:, 7:10, :],
                    op=MIN,
                )
                nc.vector.tensor_tensor(
                    out=v3[:, 4:8, :], in0=w2_3[:, 4:8, :], in1=x3[:, 8:12, :],
                    op=MIN,
                )
                if k == N - 1:
                    # shorter drain: two 2-seg pieces
                    horizontal(nc.vector, o3, slice(4, 6))
                    store(k, 4, 2)
                    horizontal(nc.vector, o3, slice(6, 8))
                    store(k, 6, 2)
                else:
                    horizontal(nc.vector, o3, slice(4, 8))
                    store(k, 4, 4)

                # loads + halos for stream position k+2 AFTER this
                # image's stores (keeps the in-order queues flowing)
                if k + 2 < N:
                    issue_load(k + 2)
                    issue_halos(k + 2)

    nc.compile()
    return nc


def run(mask: np.ndarray, trace: bool = False, tmpdir: str | None = None):
    assert mask.shape == (B, 1, H, W), mask.shape
    in_dtype = mask.dtype
    mask4 = np.ascontiguousarray(
        mask.reshape(B, H, W).astype(np.float32, copy=False)
    )
    if "nc" not in _CACHE:
        _CACHE["nc"] = build_nc(1)
    nc = _CACHE["nc"]
    in_maps = [
        {"mask": mask4[i * PER_CORE : (i + 1) * PER_CORE]} for i in range(N_CORES)
    ]
    res = run_bass_kernel_spmd(
        nc, in_maps, list(range(N_CORES)), trace=trace, tmpdir=tmpdir
    )
    out = np.concatenate([res.results[i]["out"] for i in range(N_CORES)], axis=0)
    return out.reshape(B, 1, H, W).astype(in_dtype, copy=False), res


def kernel(mask: np.ndarray) -> np.ndarray:
    return run(mask)[0]
